# revision 53
# baseline (speedup 1.0000x reference)
"""Cross-attention kernel for TRN2, 8 NeuronCores.

Sharding: core c -> (batch b = c//2, head-group g = c%2).  Each head-group is
8 heads = 512 of the 1024 d_model channels.  Within a core everything runs in
one fused software pipeline over 4 head-pair stages (o = 0..3):

  QT = wq_g.T @ q.T  (scale folded)       [512, 512]   (s, lq)
  KT = wk_g.T @ kv.T                      [512, 2048]  (s, lkv)
  V  = kv @ wv_g.T                        [2048, 512]  (lkv, s)   + ones col
  phase1(o), t = 0..15:
               ST = Kh.T-contract @ QT    [128, 512]   (lkv-tile, lq)
               P[t] = exp(ST)          -> bf16 SBUF [128, 16, 512] per head
  phase2(o), unit (hp, lt):  16 consecutive matmuls in ONE psum bank
               ctx[lq, 65] += P[t]_lt.T @ [Vh | 1]     (F=65 transposed form;
                                                        col 64 = softmax denom)
               C = ctx[:, 0:64] * recip(ctx[:, 64])  (DVE per-partition scalar)
  transpose C -> cT[s, lq]  (PE transpose, identity trick)
  out = cT.T @ wo_g.T                     [512, 1024]
Host sums the two head-group partials per batch and adds bo + bv@Wo.T
(the V bias commutes through softmax-weighted averaging: sum(a)=1, so
ctx = sum(a v) + bv and the bv term is a constant row added on host; the
K bias drops entirely -- a per-row constant shift of the scores cancels
in softmax).

All operands are bf16 (1 cyc/row on PE at any free size; halves DMA), psum
accumulation f32.  phase2(o-1) and the projection matmuls of stage o+1 are
hand-interleaved into phase1(o)'s t-loop so the Act engine's exp stream (the
second largest engine load) fully overlaps PE work.  A psum accumulation
group owns its whole 2KB bank (start zeroes the full zero-region), hence the
consecutive-16 structure of phase2 rather than round-robin accumulation.

Timing model notes (TimelineSim is the metric): the PE clock ramp anchors at
the FIRST matmul and never resets on the gap sizes this kernel produces, so
eight dummy matmuls right after the preamble have the ramp done (3us) before
the first real projection; DMA transfers serialize on the single DMA_ENGINES
device (~360 GB/s, ~1.46us per 512KB slice), so the input DMA order below is
the exact just-in-time consumption order of the stage-0 pipeline, kv streams
as eight full-depth 256-column slices (one slice = a complete quarter
K-projection contraction, putting the first score tile ~1.7us earlier than
co-major chunking), and the kernel tail is one copy+DMA chain, minimized by
making the final lq tile's high half four DMA-less [128]-col groups with a
single merged DMA at the end.
"""

import sys
if "/opt/trn_rl_repo" not in sys.path:
    sys.path.insert(0, "/opt/trn_rl_repo")

import numpy as np
import ml_dtypes

import concourse.bass as bass
import concourse.mybir as mybir
import concourse.tile as tile
from concourse.bass_utils import run_bass_kernel_spmd

f32 = mybir.dt.float32
bf16 = mybir.dt.bfloat16
EXP = mybir.ActivationFunctionType.Exp
IDENT = mybir.ActivationFunctionType.Identity

D = 1024        # d_model
S = 512         # per-core channel shard (8 heads x 64)
LQ = 512
LKV = 2048
CO = D // 128   # 8 contraction chunks
SO = S // 128   # 4 shard s-tiles (head pairs)
NT = LKV // 128  # 16 lkv tiles


def _split_multi_waits(nc, max_waits=1):
    """This container's walrus allows only `max_waits` sync-wait commands per
    instruction; hoist the excess into standalone EventSemaphore insts."""
    ev_id = 0
    for f in nc.m.functions:
        for bb in f.blocks:
            new = []
            changed = False
            for inst in bb.instructions:
                si = inst.sync_info
                if si is not None and si.on_wait and len(si.on_wait) > max_waits:
                    waits = list(si.on_wait)
                    for sw in waits[:-max_waits]:
                        ev = mybir.InstEventSemaphore(
                            name=f"EVSPLIT-{ev_id}", engine=inst.engine,
                            sync_info=mybir.SyncInfo(on_wait=[sw], on_update=[]))
                        ev_id += 1
                        nc.register_instruction(ev, overwrite=True)
                        new.append(ev)
                    inst.sync_info = mybir.SyncInfo(
                        on_wait=waits[-max_waits:], on_update=list(si.on_update))
                    changed = True
                new.append(inst)
            if changed:
                bb.instructions = new
    return nc


def _build():
    nc = bass.Bass(trn_type="TRN2")

    # DRAM I/O (pre-laid-out [128, outer, free] on host, bf16)
    qT = nc.dram_tensor("qT", [128, CO, LQ], bf16, kind="ExternalInput")
    kvT = nc.dram_tensor("kvT", [128, CO, LKV], bf16, kind="ExternalInput")
    # o=0 slices of Wq|Wk packed contiguously: one full-rate DMA on the
    # critical path to the first score matmul (256-col slices of wqT/wkT
    # alone pay the sub-512B DMA penalty)
    wqk0 = nc.dram_tensor("wqk0", [128, CO, 256], bf16, kind="ExternalInput")
    # o=1..3 slices of Wq|Wk packed: cols 0:384 = wq[:, :, 128:512],
    # cols 384:768 = wk[:, :, 128:512]
    wqkr = nc.dram_tensor("wqkr", [128, CO, 768], bf16, kind="ExternalInput")
    wvT = nc.dram_tensor("wvT", [128, CO, S], bf16, kind="ExternalInput")
    woT = nc.dram_tensor("woT", [128, SO, D], bf16, kind="ExternalInput")
    bq = nc.dram_tensor("bq", [128, SO], f32, kind="ExternalInput")
    ident = nc.dram_tensor("ident", [128, 128], bf16, kind="ExternalInput")
    out = nc.dram_tensor("out", [SO, 128, D], bf16, kind="ExternalOutput")

    with tile.TileContext(nc) as tc:
        with tc.tile_pool(name="wgt", bufs=1) as wgt, \
             tc.tile_pool(name="pt", bufs=2) as ptp, \
             tc.tile_pool(name="stg", bufs=4) as stg, \
             tc.tile_pool(name="ost", bufs=3) as ost, \
             tc.tile_pool(name="ps", bufs=1, space="PSUM") as ps:

            # ---- resident SBUF ----
            kv_sb = wgt.tile([128, CO, LKV], bf16, name="kv_sb")
            wqkr_sb = wgt.tile([128, CO, 768], bf16, name="wqkr_sb")
            wv_sb = wgt.tile([128, CO, S], bf16, name="wv_sb")
            wo_sb = wgt.tile([128, SO, D], bf16, name="wo_sb")
            qT_sb = wgt.tile([128, CO, LQ], bf16, name="qT_sb")
            QT_sb = wgt.tile([128, SO, LQ], bf16, name="QT_sb")
            KT_sb = wgt.tile([128, SO, LKV], bf16, name="KT_sb")
            # V per head with a ones column: attn@V (transposed form) then
            # also yields the softmax denominator in output col 64.
            Vp_sb = wgt.tile([128, NT, 8, 65], bf16, name="Vp_sb")
            cT_sb = wgt.tile([128, SO, LQ], bf16, name="cT_sb")
            bq_sb = wgt.tile([128, SO], f32, name="bq_sb")
            ident_sb = wgt.tile([128, 128], bf16, name="ident_sb")
            wqk0_sb = wgt.tile([128, CO, 256], bf16, name="wqk0_sb")

            # ---- PE clock-ramp anchor: the ramp timer starts at the first
            # matmul and survives the idle gaps this kernel produces, so a
            # few cheap dummies right after the preamble put the engine at
            # full clock (3us later) before the first real projection.
            dm_sb = wgt.tile([128, 512], bf16, name="dm_sb")
            nc.vector.memset(dm_sb, 0.0)
            for i in range(8):
                dps = ps.tile([128, 512], f32, name=f"dps{i}", tag="proj",
                              bufs=2)
                nc.tensor.matmul(dps, dm_sb[:, 0:128], dm_sb,
                                 start=True, stop=True)

            # ---- DMA order = just-in-time consumption order (transfers
            # serialize on the DMA_ENGINES device).  Stage 0 needs the o=0
            # weight slices, q, and the kv chunks in score order; weights for
            # later stages and the out-projection arrive behind them.
            nc.sync.dma_start(wqk0_sb, wqk0[:])
            nc.sync.dma_start(qT_sb[:, 0:4, :], qT[:, 0:4, :])
            nc.sync.dma_start(qT_sb[:, 4:8, :], qT[:, 4:8, :])
            # kv as eight full-depth 256-column slices: one slice is a
            # complete contraction for a quarter of the K projection, so the
            # first score tile is gated by ONE kv transfer instead of two,
            # and each following slice feeds exactly two score tiles.
            nc.sync.dma_start(kv_sb[:, :, 0:256], kvT[:, :, 0:256])
            nc.sync.dma_start(bq_sb, bq[:])
            for sl8 in range(1, 6):
                nc.sync.dma_start(
                    kv_sb[:, :, sl8 * 256:(sl8 + 1) * 256],
                    kvT[:, :, sl8 * 256:(sl8 + 1) * 256])
            # wv's first half lands before the last two kv slices: the V
            # projections (wv cols 0:128) start at t=9, before the t=12/14
            # score tiles that need kv slices 6-7
            nc.sync.dma_start(wv_sb[:, :, 0:256], wvT[:, :, 0:256])
            for sl8 in range(6, 8):
                nc.sync.dma_start(
                    kv_sb[:, :, sl8 * 256:(sl8 + 1) * 256],
                    kvT[:, :, sl8 * 256:(sl8 + 1) * 256])
            nc.sync.dma_start(wv_sb[:, :, 256:512], wvT[:, :, 256:512])
            nc.sync.dma_start(ident_sb, ident[:])
            nc.sync.dma_start(wqkr_sb, wqkr[:])
            nc.sync.dma_start(wo_sb, woT[:])

            nc.vector.memset(Vp_sb[:, :, :, 64:65], 1.0)

            # ---- emission helpers (each emits PE matmuls + its drain) ----
            def kproj(o, ch):
                kps = ps.tile([128, 512], f32, name=f"kps{o}_{ch}",
                              tag="proj", bufs=2)
                sl = slice(ch * 512, (ch + 1) * 512)
                for c in range(CO):
                    w = (wqk0_sb[:, c, 128:256] if o == 0 else
                         wqkr_sb[:, c, 384 + (o - 1) * 128:384 + o * 128])
                    nc.tensor.matmul(kps, w, kv_sb[:, c, sl],
                                     start=(c == 0), stop=(c == CO - 1))
                nc.vector.tensor_copy(KT_sb[:, o, sl], kps)

            def kprojQ(q):
                """Stage-0 K projection, one 256-column quarter: a complete
                8-chunk contraction over a single kv column-slice, so it is
                ready one DMA after that slice lands.  Quarter 0 drains in
                two half-copies: the very first score tile only needs its
                own 128 columns, and its wait chain is the t0 gate."""
                kps = ps.tile([128, 256], f32, name=f"kpsq{q}", tag="proj",
                              bufs=2)
                sl = slice(q * 256, (q + 1) * 256)
                for c in range(CO):
                    nc.tensor.matmul(kps, wqk0_sb[:, c, 128:256],
                                     kv_sb[:, c, sl],
                                     start=(c == 0), stop=(c == CO - 1))
                if q == 0:
                    nc.vector.tensor_copy(KT_sb[:, 0, 0:128], kps[:, 0:128])
                    nc.vector.tensor_copy(KT_sb[:, 0, 128:256],
                                          kps[:, 128:256])
                else:
                    nc.vector.tensor_copy(KT_sb[:, 0, sl], kps)

            def qproj(o):
                qps = ps.tile([128, 512], f32, name=f"qps{o}", tag="proj",
                              bufs=2)
                for c in range(CO):
                    w = (wqk0_sb[:, c, 0:128] if o == 0 else
                         wqkr_sb[:, c, (o - 1) * 128:o * 128])
                    nc.tensor.matmul(qps, w, qT_sb[:, c, :],
                                     start=(c == 0), stop=(c == CO - 1))
                nc.vector.tensor_scalar_add(QT_sb[:, o, :], qps,
                                            bq_sb[:, o:o + 1])

            def vproj(o, t):
                vps = ps.tile([128, 128], f32, name=f"vps{o}_{t}", tag="proj",
                              bufs=2)
                tsl = slice(t * 128, (t + 1) * 128)
                osl = slice(o * 128, (o + 1) * 128)
                for c in range(CO):
                    nc.tensor.matmul(vps, kv_sb[:, c, tsl], wv_sb[:, c, osl],
                                     start=(c == 0), stop=(c == CO - 1))
                nc.vector.tensor_copy(
                    Vp_sb[:, t, 2 * o:2 * o + 2, 0:64],
                    vps.rearrange("p (h d) -> p h d", h=2))

            # ---- lead-in: stage-0 prerequisites ----
            qproj(0)
            kprojQ(0)

            # Per-stage fill schedules: iteration t -> thunks.  Placement
            # matches DMA arrival order (PE is in-order, so emitting a matmul
            # whose DMA lands late would stall everything behind it).
            def mk_sched(o):
                s = {t: [] for t in range(NT)}
                if o == 0:
                    # kv column-slices land one per ~1.5us; quarter-pass q
                    # feeds the two score tiles at t=2q, so emit it two
                    # tiles ahead, just-in-time with its slice's arrival.
                    for q in range(1, 8):
                        s[2 * (q - 1)].append(lambda q=q: kprojQ(q))
                    nv = 0
                    for t in range(9, NT):
                        take = 2 if t < 14 else 3
                        for _ in range(take):
                            if nv < NT:
                                s[t].append(lambda v=nv: vproj(0, v))
                                nv += 1
                else:
                    # own K chunks 1..3 first (ch0/qproj ran at the tail of
                    # the previous stage), V tiles just-in-time for phase2.
                    for ch in range(1, 4):
                        s[ch - 1].append(lambda ch=ch: kproj(o, ch))
                    for t in range(NT):
                        s[t].append(lambda t=t: vproj(o, t))
                if o < 3:
                    # next stage's first K chunk before its Q projection:
                    # the next stage's first score tile is gated by the
                    # KT-copy drain, so give it the extra headroom
                    s[NT - 2].append(lambda: kproj(o + 1, 0))
                    s[NT - 1].append(lambda: qproj(o + 1))
                return s

            def phase2_unit(o, pt, hp, lt, c_sb):
                """ctx unit (head hp of pair o, lq tile lt): 16 consecutive
                matmuls in one psum bank, then normalize straight from psum.
                (An accumulation group owns its whole 2KB zero-region, so the
                16 steps must be consecutive in one dedicated bank.)
                Pair 3 runs at the kernel tail where Act is idle, so its
                normalize goes to the scalar engine instead of DVE, and its
                units alternate over the then-idle proj banks as well to keep
                4 accumulations in flight instead of 2."""
                if o == SO - 1:
                    # score banks are idle after stage 3's last exp: rotate
                    # over ctx/st (4 buffers) so accumulations stay in
                    # flight; proj stays exclusive to the transposes
                    tag = ("ctx", "st")[(hp * SO + lt) % 2]
                else:
                    tag = "ctx"
                ctx = ps.tile([128, 65], f32, name=f"ctx{o}_{hp}_{lt}",
                              tag=tag, bufs=2)
                base = hp * 512 + lt * 128
                for t in range(NT):
                    nc.tensor.matmul(
                        ctx, pt[:, t, base:base + 128],
                        Vp_sb[:, t, 2 * o + hp, :],
                        start=(t == 0), stop=(t == NT - 1))
                rc = stg.tile([128, 1], f32, name=f"rc{o}_{hp}_{lt}", tag="rc",
                              bufs=4)
                nc.vector.reciprocal(rc, ctx[:, 64:65])
                if o == SO - 1:
                    nc.scalar.activation(c_sb[:, lt, hp, :], ctx[:, 0:64],
                                         IDENT, scale=rc)
                else:
                    nc.vector.tensor_scalar_mul(
                        c_sb[:, lt, hp, :], ctx[:, 0:64], rc)

            def transpose_lt(o, lt, c_sb):
                """One 128-partition transpose covers BOTH heads of the
                pair for lq-tile lt: c_sb[:, (hp,s), lt] is [128, 128] and
                its transpose is exactly cT's [s-part (hp-major), lq]
                layout -- half the PE rows of two per-head transposes.  The
                psum tile is drained in the same step so it never blocks
                the proj-tag rotation of the interleaved projections."""
                trp = ps.tile([128, 128], bf16, name=f"trp{o}_{lt}",
                              tag="proj", bufs=2)
                nc.tensor.transpose(
                    trp, c_sb[:, lt, :, :].rearrange("p h d -> p (h d)"),
                    ident_sb)
                nc.vector.tensor_copy(
                    cT_sb[:, o, lt * 128:(lt + 1) * 128], trp)

            def phase2_steps(o, pt):
                """Thunks: 8 ctx units (lt-major) + 4 fused transposes for
                pair-stage o."""
                c_sb = stg.tile([128, SO, 2, 64], bf16, name=f"c{o}", tag="c",
                                bufs=2)
                for hp in range(2):
                    for lt in range(SO):
                        yield lambda hp=hp, lt=lt: phase2_unit(
                            o, pt, hp, lt, c_sb)
                for lt in range(SO):
                    yield lambda lt=lt: transpose_lt(o, lt, c_sb)

            # ---- 4 head-pair stages ----
            prev_p2 = None   # phase2 step iterator of the previous stage
            for o in range(SO):
                sched = mk_sched(o)
                pt = ptp.tile([128, NT, 1024], bf16, name=f"pt{o}",
                              tag="pt", bufs=2)
                for t in range(NT):
                    # fused score tile: head 2o in bank cols 0:512, head
                    # 2o+1 in 512:1024 (each matmul stays within one bank)
                    st2 = ps.tile([128, 1024], f32, name=f"st{o}_{t}",
                                  tag="st", bufs=2)
                    tsl = slice(t * 128, (t + 1) * 128)
                    nc.tensor.matmul(st2[:, 0:512], KT_sb[0:64, o, tsl],
                                     QT_sb[0:64, o, :], start=True, stop=True)
                    nc.tensor.matmul(st2[:, 512:1024], KT_sb[64:128, o, tsl],
                                     QT_sb[64:128, o, :], start=True, stop=True)
                    nc.scalar.activation(pt[:, t, :], st2, EXP)
                    # one phase2 step of the previous stage every other t,
                    # plus two extra slots so only 2 of the 12 steps bunch
                    # up at the stage-end drain
                    if (t % 2 == 1 or t in (8, 12)) and prev_p2 is not None:
                        step = next(prev_p2, None)
                        if step is not None:
                            step()
                        if t == NT - 1:  # 12 steps total, drain leftovers
                            for step in prev_p2:
                                step()
                    for thunk in sched[t]:
                        thunk()
                if o < SO - 1:
                    prev_p2 = phase2_steps(o, pt)

            # ---- final drain: stage-3 phase2 woven with the out
            # projection (out[lq, d] += cT[:, o, lq-sl].T @ wo).  Each lq
            # tile's out-projection groups follow its own fused transpose,
            # so the normalize->transpose chains of the last head-pair hide
            # behind ~1.7us of out-projection matmuls per tile.  The DMA
            # tail stays a single short chain: the kernel end is gated by
            # sem(~200) + copy + HWDGE(625, single slot) + DGE(650) +
            # transfer + 900, and any other DMA issued within ~1.3us before
            # it would queue ahead of it on HWDGE and become the laggard --
            # hence lt3's high half as four DMA-less [128]-col groups
            # (~0.9us of PE work) with one merged DMA at the very end.
            c3 = stg.tile([128, SO, 2, 64], bf16, name="c3", tag="c",
                          bufs=2)
            ot_tiles = {}
            for lt in range(SO):
                ot_tiles[lt] = ost.tile([128, D], bf16, name=f"ot{lt}",
                                        tag=f"ot{lt}")
            gi_n = [0]

            def ogroup(lt, c0, c1, eng, dma, tag):
                gi = gi_n[0]
                gi_n[0] += 1
                ops = ps.tile([128, c1 - c0], f32, name=f"ops{gi}",
                              tag=tag, bufs=2)
                lsl = slice(lt * 128, (lt + 1) * 128)
                for o in range(SO):
                    nc.tensor.matmul(ops, cT_sb[:, o, lsl],
                                     wo_sb[:, o, c0:c1],
                                     start=(o == 0), stop=(o == SO - 1))
                ot = ot_tiles[lt]
                if eng == "act":
                    nc.scalar.activation(ot[:, c0:c1], ops, IDENT)
                else:
                    nc.vector.tensor_copy(ot[:, c0:c1], ops)
                if dma == "piece":
                    nc.sync.dma_start(out[lt, :, c0:c1], ot[:, c0:c1])
                elif dma == "full":
                    nc.sync.dma_start(out[lt, :, :], ot)
                elif dma == "tail":
                    # covers all four [128]-col groups; waits all copies
                    nc.sync.dma_start(out[lt, :, 512:1024], ot[:, 512:1024])

            for hp in range(2):
                for lt in range(SO):
                    phase2_unit(3, pt, hp, lt, c3)
            for lt in range(SO):
                transpose_lt(3, lt, c3)
            ogroup(3, 0, 512, "act", "piece", "proj")
            for lt in range(3):
                # lt2 ships as two piece-DMAs: its high half (the last big
                # transfer before the merged tail DMA) is then 364ns instead
                # of 790, letting the tail transfer start earlier; its copy
                # goes to Act so the piece-DMA's SEQ hold starts (and ends)
                # before the merged tail DMA needs the queue
                ogroup(lt, 0, 512, "act",
                       "piece" if lt == 2 else None, "st")
                ogroup(lt, 512, 1024, "act" if lt == 2 else "dve",
                       "piece" if lt == 2 else "full", "proj")
            for i in range(4):
                ogroup(3, 512 + 128 * i, 640 + 128 * i,
                       ("act", "dve")[i % 2],
                       "tail" if i == 3 else None,
                       ("st", "ctx", "proj", "ctx")[i])

    return _split_multi_waits(nc)


_NC = None


def _get_nc():
    global _NC
    if _NC is None:
        _NC = _build()
    return _NC


def _shard(q, kv, Wq, bq, Wk, bk, Wv, bv, Wo, bo):
    b16 = ml_dtypes.bfloat16

    def lay(a2d, co):  # [co*128, F] -> [128, co, F]
        F = a2d.shape[1]
        return np.ascontiguousarray(
            a2d.reshape(co, 128, F).transpose(1, 0, 2)).astype(b16)

    idn = np.eye(128, dtype=b16)
    in_maps = []
    for core in range(8):
        b, g = core // 2, core % 2
        sl = slice(g * S, (g + 1) * S)
        wq_l = lay(np.ascontiguousarray((Wq[sl] * 0.125).T), CO)
        wk_l = lay(np.ascontiguousarray(Wk[sl].T), CO)
        m = {
            "wqk0": np.ascontiguousarray(
                np.concatenate([wq_l[:, :, 0:128], wk_l[:, :, 0:128]],
                               axis=2)),
            "wqkr": np.ascontiguousarray(
                np.concatenate([wq_l[:, :, 128:512], wk_l[:, :, 128:512]],
                               axis=2)),
            "qT": lay(np.ascontiguousarray(q[b].T), CO),
            "kvT": lay(np.ascontiguousarray(kv[b].T), CO),
            "wvT": lay(np.ascontiguousarray(Wv[sl].T), CO),
            "woT": lay(np.ascontiguousarray(Wo[:, sl].T), SO),
            "bq": np.ascontiguousarray(
                (bq[sl] * 0.125).reshape(SO, 128).T).astype(np.float32),
            "ident": idn,
        }
        in_maps.append(m)
    return in_maps


def _run(in_maps, trace=False):
    res = run_bass_kernel_spmd(_get_nc(), in_maps, core_ids=list(range(8)),
                               trace=trace)
    return res


def kernel(q, kv, Wq, bq, Wk, bk, Wv, bv, Wo, bo, _trace=False):
    q, kv = np.asarray(q, np.float32), np.asarray(kv, np.float32)
    Wq, Wk = np.asarray(Wq, np.float32), np.asarray(Wk, np.float32)
    Wv, Wo = np.asarray(Wv, np.float32), np.asarray(Wo, np.float32)
    bq, bk = np.asarray(bq, np.float32), np.asarray(bk, np.float32)
    bv, bo = np.asarray(bv, np.float32), np.asarray(bo, np.float32)

    in_maps = _shard(q, kv, Wq, bq, Wk, bk, Wv, bv, Wo, bo)
    res = _run(in_maps, trace=_trace)
    B = q.shape[0]
    # bv commutes through the softmax average; bk cancels in softmax.
    const_row = bv @ Wo.T + bo
    outp = np.empty((B, LQ, D), np.float32)
    for b in range(B):
        p0 = np.asarray(res.results[2 * b]["out"],
                        np.float32).reshape(LQ, D)
        p1 = np.asarray(res.results[2 * b + 1]["out"],
                        np.float32).reshape(LQ, D)
        outp[b] = p0 + p1 + const_row[None, :]
    if _trace:
        kernel._last_exec_ns = res.exec_time_ns
        kernel._last_trace = res.instructions_and_trace
    return outp


# revision 54
# speedup vs baseline: 1.0001x; 1.0001x over previous
"""Cross-attention kernel for TRN2, 8 NeuronCores.

Sharding: core c -> (batch b = c//2, head-group g = c%2).  Each head-group is
8 heads = 512 of the 1024 d_model channels.  Within a core everything runs in
one fused software pipeline over 4 head-pair stages (o = 0..3):

  QT = wq_g.T @ q.T  (scale folded)       [512, 512]   (s, lq)
  KT = wk_g.T @ kv.T                      [512, 2048]  (s, lkv)
  V  = kv @ wv_g.T                        [2048, 512]  (lkv, s)   + ones col
  phase1(o), t = 0..15:
               ST = Kh.T-contract @ QT    [128, 512]   (lkv-tile, lq)
               P[t] = exp(ST)          -> bf16 SBUF [128, 16, 512] per head
  phase2(o), unit (hp, lt):  16 consecutive matmuls in ONE psum bank
               ctx[lq, 65] += P[t]_lt.T @ [Vh | 1]     (F=65 transposed form;
                                                        col 64 = softmax denom)
               C = ctx[:, 0:64] * recip(ctx[:, 64])  (DVE per-partition scalar)
  transpose C -> cT[s, lq]  (PE transpose, identity trick)
  out = cT.T @ wo_g.T                     [512, 1024]
Host sums the two head-group partials per batch and adds bo + bv@Wo.T
(the V bias commutes through softmax-weighted averaging: sum(a)=1, so
ctx = sum(a v) + bv and the bv term is a constant row added on host; the
K bias drops entirely -- a per-row constant shift of the scores cancels
in softmax).

All operands are bf16 (1 cyc/row on PE at any free size; halves DMA), psum
accumulation f32.  phase2(o-1) and the projection matmuls of stage o+1 are
hand-interleaved into phase1(o)'s t-loop so the Act engine's exp stream (the
second largest engine load) fully overlaps PE work.  A psum accumulation
group owns its whole 2KB bank (start zeroes the full zero-region), hence the
consecutive-16 structure of phase2 rather than round-robin accumulation.

Timing model notes (TimelineSim is the metric): the PE clock ramp anchors at
the FIRST matmul and never resets on the gap sizes this kernel produces, so
eight dummy matmuls right after the preamble have the ramp done (3us) before
the first real projection; DMA transfers serialize on the single DMA_ENGINES
device (~360 GB/s, ~1.46us per 512KB slice), so the input DMA order below is
the exact just-in-time consumption order of the stage-0 pipeline, kv streams
as eight full-depth 256-column slices (one slice = a complete quarter
K-projection contraction, putting the first score tile ~1.7us earlier than
co-major chunking), and the kernel tail is one copy+DMA chain, minimized by
making the final lq tile's high half four DMA-less [128]-col groups with a
single merged DMA at the end.
"""

import sys
if "/opt/trn_rl_repo" not in sys.path:
    sys.path.insert(0, "/opt/trn_rl_repo")

import numpy as np
import ml_dtypes

import concourse.bass as bass
import concourse.mybir as mybir
import concourse.tile as tile
from concourse.bass_utils import run_bass_kernel_spmd

f32 = mybir.dt.float32
bf16 = mybir.dt.bfloat16
EXP = mybir.ActivationFunctionType.Exp
IDENT = mybir.ActivationFunctionType.Identity

D = 1024        # d_model
S = 512         # per-core channel shard (8 heads x 64)
LQ = 512
LKV = 2048
CO = D // 128   # 8 contraction chunks
SO = S // 128   # 4 shard s-tiles (head pairs)
NT = LKV // 128  # 16 lkv tiles


def _split_multi_waits(nc, max_waits=1):
    """This container's walrus allows only `max_waits` sync-wait commands per
    instruction; hoist the excess into standalone EventSemaphore insts."""
    ev_id = 0
    for f in nc.m.functions:
        for bb in f.blocks:
            new = []
            changed = False
            for inst in bb.instructions:
                si = inst.sync_info
                if si is not None and si.on_wait and len(si.on_wait) > max_waits:
                    waits = list(si.on_wait)
                    for sw in waits[:-max_waits]:
                        ev = mybir.InstEventSemaphore(
                            name=f"EVSPLIT-{ev_id}", engine=inst.engine,
                            sync_info=mybir.SyncInfo(on_wait=[sw], on_update=[]))
                        ev_id += 1
                        nc.register_instruction(ev, overwrite=True)
                        new.append(ev)
                    inst.sync_info = mybir.SyncInfo(
                        on_wait=waits[-max_waits:], on_update=list(si.on_update))
                    changed = True
                new.append(inst)
            if changed:
                bb.instructions = new
    return nc


def _build():
    nc = bass.Bass(trn_type="TRN2")

    # DRAM I/O (pre-laid-out [128, outer, free] on host, bf16)
    qT = nc.dram_tensor("qT", [128, CO, LQ], bf16, kind="ExternalInput")
    kvT = nc.dram_tensor("kvT", [128, CO, LKV], bf16, kind="ExternalInput")
    # o=0 slices of Wq|Wk packed contiguously: one full-rate DMA on the
    # critical path to the first score matmul (256-col slices of wqT/wkT
    # alone pay the sub-512B DMA penalty)
    wqk0 = nc.dram_tensor("wqk0", [128, CO, 256], bf16, kind="ExternalInput")
    # o=1..3 slices of Wq|Wk packed: cols 0:384 = wq[:, :, 128:512],
    # cols 384:768 = wk[:, :, 128:512]
    wqkr = nc.dram_tensor("wqkr", [128, CO, 768], bf16, kind="ExternalInput")
    wvT = nc.dram_tensor("wvT", [128, CO, S], bf16, kind="ExternalInput")
    woT = nc.dram_tensor("woT", [128, SO, D], bf16, kind="ExternalInput")
    bq = nc.dram_tensor("bq", [128, SO], f32, kind="ExternalInput")
    ident = nc.dram_tensor("ident", [128, 128], bf16, kind="ExternalInput")
    out = nc.dram_tensor("out", [SO, 128, D], bf16, kind="ExternalOutput")

    with tile.TileContext(nc) as tc:
        with tc.tile_pool(name="wgt", bufs=1) as wgt, \
             tc.tile_pool(name="pt", bufs=2) as ptp, \
             tc.tile_pool(name="stg", bufs=4) as stg, \
             tc.tile_pool(name="ost", bufs=3) as ost, \
             tc.tile_pool(name="ps", bufs=1, space="PSUM") as ps:

            # ---- resident SBUF ----
            kv_sb = wgt.tile([128, CO, LKV], bf16, name="kv_sb")
            wqkr_sb = wgt.tile([128, CO, 768], bf16, name="wqkr_sb")
            wv_sb = wgt.tile([128, CO, S], bf16, name="wv_sb")
            wo_sb = wgt.tile([128, SO, D], bf16, name="wo_sb")
            qT_sb = wgt.tile([128, CO, LQ], bf16, name="qT_sb")
            QT_sb = wgt.tile([128, SO, LQ], bf16, name="QT_sb")
            KT_sb = wgt.tile([128, SO, LKV], bf16, name="KT_sb")
            # V per head with a ones column: attn@V (transposed form) then
            # also yields the softmax denominator in output col 64.
            Vp_sb = wgt.tile([128, NT, 8, 65], bf16, name="Vp_sb")
            cT_sb = wgt.tile([128, SO, LQ], bf16, name="cT_sb")
            bq_sb = wgt.tile([128, SO], f32, name="bq_sb")
            ident_sb = wgt.tile([128, 128], bf16, name="ident_sb")
            wqk0_sb = wgt.tile([128, CO, 256], bf16, name="wqk0_sb")

            # ---- PE clock-ramp anchor: the ramp timer starts at the first
            # matmul and survives the idle gaps this kernel produces, so a
            # few cheap dummies right after the preamble put the engine at
            # full clock (3us later) before the first real projection.
            dm_sb = wgt.tile([128, 512], bf16, name="dm_sb")
            nc.vector.memset(dm_sb, 0.0)
            for i in range(8):
                dps = ps.tile([128, 512], f32, name=f"dps{i}", tag="proj",
                              bufs=2)
                nc.tensor.matmul(dps, dm_sb[:, 0:128], dm_sb,
                                 start=True, stop=True)

            # ---- DMA order = just-in-time consumption order (transfers
            # serialize on the DMA_ENGINES device).  Stage 0 needs the o=0
            # weight slices, q, and the kv chunks in score order; weights for
            # later stages and the out-projection arrive behind them.
            nc.sync.dma_start(wqk0_sb, wqk0[:])
            nc.sync.dma_start(qT_sb[:, 0:4, :], qT[:, 0:4, :])
            nc.sync.dma_start(qT_sb[:, 4:8, :], qT[:, 4:8, :])
            # kv as eight full-depth 256-column slices: one slice is a
            # complete contraction for a quarter of the K projection, so the
            # first score tile is gated by ONE kv transfer instead of two,
            # and each following slice feeds exactly two score tiles.
            nc.sync.dma_start(kv_sb[:, :, 0:256], kvT[:, :, 0:256])
            nc.sync.dma_start(bq_sb, bq[:])
            for sl8 in range(1, 6):
                nc.sync.dma_start(
                    kv_sb[:, :, sl8 * 256:(sl8 + 1) * 256],
                    kvT[:, :, sl8 * 256:(sl8 + 1) * 256])
            # wv's first half lands before the last two kv slices: the V
            # projections (wv cols 0:128) start at t=9, before the t=12/14
            # score tiles that need kv slices 6-7
            nc.sync.dma_start(wv_sb[:, :, 0:256], wvT[:, :, 0:256])
            for sl8 in range(6, 8):
                nc.sync.dma_start(
                    kv_sb[:, :, sl8 * 256:(sl8 + 1) * 256],
                    kvT[:, :, sl8 * 256:(sl8 + 1) * 256])
            nc.sync.dma_start(wv_sb[:, :, 256:512], wvT[:, :, 256:512])
            nc.sync.dma_start(ident_sb, ident[:])
            nc.sync.dma_start(wqkr_sb, wqkr[:])
            nc.sync.dma_start(wo_sb, woT[:])

            nc.vector.memset(Vp_sb[:, :, :, 64:65], 1.0)

            # ---- emission helpers (each emits PE matmuls + its drain) ----
            def kproj(o, ch):
                kps = ps.tile([128, 512], f32, name=f"kps{o}_{ch}",
                              tag="proj", bufs=2)
                sl = slice(ch * 512, (ch + 1) * 512)
                for c in range(CO):
                    w = (wqk0_sb[:, c, 128:256] if o == 0 else
                         wqkr_sb[:, c, 384 + (o - 1) * 128:384 + o * 128])
                    nc.tensor.matmul(kps, w, kv_sb[:, c, sl],
                                     start=(c == 0), stop=(c == CO - 1))
                nc.vector.tensor_copy(KT_sb[:, o, sl], kps)

            def kprojQ(q):
                """Stage-0 K projection, one 256-column quarter: a complete
                8-chunk contraction over a single kv column-slice, so it is
                ready one DMA after that slice lands.  Quarter 0 drains in
                two half-copies: the very first score tile only needs its
                own 128 columns, and its wait chain is the t0 gate."""
                kps = ps.tile([128, 256], f32, name=f"kpsq{q}", tag="proj",
                              bufs=2)
                sl = slice(q * 256, (q + 1) * 256)
                for c in range(CO):
                    nc.tensor.matmul(kps, wqk0_sb[:, c, 128:256],
                                     kv_sb[:, c, sl],
                                     start=(c == 0), stop=(c == CO - 1))
                if q == 0:
                    nc.vector.tensor_copy(KT_sb[:, 0, 0:128], kps[:, 0:128])
                    nc.vector.tensor_copy(KT_sb[:, 0, 128:256],
                                          kps[:, 128:256])
                else:
                    nc.vector.tensor_copy(KT_sb[:, 0, sl], kps)

            def qproj(o):
                qps = ps.tile([128, 512], f32, name=f"qps{o}", tag="proj",
                              bufs=2)
                for c in range(CO):
                    w = (wqk0_sb[:, c, 0:128] if o == 0 else
                         wqkr_sb[:, c, (o - 1) * 128:o * 128])
                    nc.tensor.matmul(qps, w, qT_sb[:, c, :],
                                     start=(c == 0), stop=(c == CO - 1))
                nc.vector.tensor_scalar_add(QT_sb[:, o, :], qps,
                                            bq_sb[:, o:o + 1])

            def vproj(o, t):
                vps = ps.tile([128, 128], f32, name=f"vps{o}_{t}", tag="proj",
                              bufs=2)
                tsl = slice(t * 128, (t + 1) * 128)
                osl = slice(o * 128, (o + 1) * 128)
                for c in range(CO):
                    nc.tensor.matmul(vps, kv_sb[:, c, tsl], wv_sb[:, c, osl],
                                     start=(c == 0), stop=(c == CO - 1))
                nc.vector.tensor_copy(
                    Vp_sb[:, t, 2 * o:2 * o + 2, 0:64],
                    vps.rearrange("p (h d) -> p h d", h=2))

            # ---- lead-in: stage-0 prerequisites ----
            qproj(0)
            kprojQ(0)

            # Per-stage fill schedules: iteration t -> thunks.  Placement
            # matches DMA arrival order (PE is in-order, so emitting a matmul
            # whose DMA lands late would stall everything behind it).
            def mk_sched(o):
                s = {t: [] for t in range(NT)}
                if o == 0:
                    # kv column-slices land one per ~1.5us; quarter-pass q
                    # feeds the two score tiles at t=2q, so emit it two
                    # tiles ahead, just-in-time with its slice's arrival.
                    for q in range(1, 8):
                        s[2 * (q - 1)].append(lambda q=q: kprojQ(q))
                    nv = 0
                    for t in range(9, NT):
                        take = 2 if t < 14 else 3
                        for _ in range(take):
                            if nv < NT:
                                s[t].append(lambda v=nv: vproj(0, v))
                                nv += 1
                else:
                    # own K chunks 1..3 first (ch0/qproj ran at the tail of
                    # the previous stage), V tiles just-in-time for phase2.
                    for ch in range(1, 4):
                        s[ch - 1].append(lambda ch=ch: kproj(o, ch))
                    for t in range(NT):
                        s[t].append(lambda t=t: vproj(o, t))
                if o < 3:
                    # next stage's first K chunk before its Q projection:
                    # the next stage's first score tile is gated by the
                    # KT-copy drain, so give it the extra headroom
                    s[NT - 2].append(lambda: kproj(o + 1, 0))
                    s[NT - 1].append(lambda: qproj(o + 1))
                return s

            def phase2_unit(o, pt, hp, lt, c_sb):
                """ctx unit (head hp of pair o, lq tile lt): 16 consecutive
                matmuls in one psum bank, then normalize straight from psum.
                (An accumulation group owns its whole 2KB zero-region, so the
                16 steps must be consecutive in one dedicated bank.)
                Pair 3 runs at the kernel tail where Act is idle, so its
                normalize goes to the scalar engine instead of DVE, and its
                units alternate over the then-idle proj banks as well to keep
                4 accumulations in flight instead of 2."""
                if o == SO - 1:
                    # score banks are idle after stage 3's last exp: rotate
                    # over ctx/st (4 buffers) so accumulations stay in
                    # flight; proj stays exclusive to the transposes
                    tag = ("ctx", "st")[(hp * SO + lt) % 2]
                else:
                    tag = "ctx"
                ctx = ps.tile([128, 65], f32, name=f"ctx{o}_{hp}_{lt}",
                              tag=tag, bufs=2)
                base = hp * 512 + lt * 128
                for t in range(NT):
                    nc.tensor.matmul(
                        ctx, pt[:, t, base:base + 128],
                        Vp_sb[:, t, 2 * o + hp, :],
                        start=(t == 0), stop=(t == NT - 1))
                rc = stg.tile([128, 1], f32, name=f"rc{o}_{hp}_{lt}", tag="rc",
                              bufs=4)
                nc.vector.reciprocal(rc, ctx[:, 64:65])
                if o == SO - 1:
                    nc.scalar.activation(c_sb[:, lt, hp, :], ctx[:, 0:64],
                                         IDENT, scale=rc)
                else:
                    nc.vector.tensor_scalar_mul(
                        c_sb[:, lt, hp, :], ctx[:, 0:64], rc)

            def transpose_lt(o, lt, c_sb):
                """One 128-partition transpose covers BOTH heads of the
                pair for lq-tile lt: c_sb[:, (hp,s), lt] is [128, 128] and
                its transpose is exactly cT's [s-part (hp-major), lq]
                layout -- half the PE rows of two per-head transposes.  The
                psum tile is drained in the same step so it never blocks
                the proj-tag rotation of the interleaved projections."""
                trp = ps.tile([128, 128], bf16, name=f"trp{o}_{lt}",
                              tag="proj", bufs=2)
                nc.tensor.transpose(
                    trp, c_sb[:, lt, :, :].rearrange("p h d -> p (h d)"),
                    ident_sb)
                nc.vector.tensor_copy(
                    cT_sb[:, o, lt * 128:(lt + 1) * 128], trp)

            def phase2_steps(o, pt):
                """Thunks: 8 ctx units (lt-major) + 4 fused transposes for
                pair-stage o."""
                c_sb = stg.tile([128, SO, 2, 64], bf16, name=f"c{o}", tag="c",
                                bufs=2)
                for hp in range(2):
                    for lt in range(SO):
                        yield lambda hp=hp, lt=lt: phase2_unit(
                            o, pt, hp, lt, c_sb)
                for lt in range(SO):
                    yield lambda lt=lt: transpose_lt(o, lt, c_sb)

            # ---- 4 head-pair stages ----
            prev_p2 = None   # phase2 step iterator of the previous stage
            for o in range(SO):
                sched = mk_sched(o)
                pt = ptp.tile([128, NT, 1024], bf16, name=f"pt{o}",
                              tag="pt", bufs=2)
                for t in range(NT):
                    # fused score tile: head 2o in bank cols 0:512, head
                    # 2o+1 in 512:1024 (each matmul stays within one bank)
                    st2 = ps.tile([128, 1024], f32, name=f"st{o}_{t}",
                                  tag="st", bufs=2)
                    tsl = slice(t * 128, (t + 1) * 128)
                    nc.tensor.matmul(st2[:, 0:512], KT_sb[0:64, o, tsl],
                                     QT_sb[0:64, o, :], start=True, stop=True)
                    nc.tensor.matmul(st2[:, 512:1024], KT_sb[64:128, o, tsl],
                                     QT_sb[64:128, o, :], start=True, stop=True)
                    nc.scalar.activation(pt[:, t, :], st2, EXP)
                    # one phase2 step of the previous stage every other t,
                    # plus two extra slots so only 2 of the 12 steps bunch
                    # up at the stage-end drain
                    if (t % 2 == 1 or t in (8, 12)) and prev_p2 is not None:
                        step = next(prev_p2, None)
                        if step is not None:
                            step()
                        if t == NT - 1:  # 12 steps total, drain leftovers
                            for step in prev_p2:
                                step()
                    for thunk in sched[t]:
                        thunk()
                if o < SO - 1:
                    prev_p2 = phase2_steps(o, pt)

            # ---- final drain: stage-3 phase2 woven with the out
            # projection (out[lq, d] += cT[:, o, lq-sl].T @ wo).  Each lq
            # tile's out-projection groups follow its own fused transpose,
            # so the normalize->transpose chains of the last head-pair hide
            # behind ~1.7us of out-projection matmuls per tile.  The DMA
            # tail stays a single short chain: the kernel end is gated by
            # sem(~200) + copy + HWDGE(625, single slot) + DGE(650) +
            # transfer + 900, and any other DMA issued within ~1.3us before
            # it would queue ahead of it on HWDGE and become the laggard --
            # hence lt3's high half as four DMA-less [128]-col groups
            # (~0.9us of PE work) with one merged DMA at the very end.
            c3 = stg.tile([128, SO, 2, 64], bf16, name="c3", tag="c",
                          bufs=2)
            ot_tiles = {}
            for lt in range(SO):
                ot_tiles[lt] = ost.tile([128, D], bf16, name=f"ot{lt}",
                                        tag=f"ot{lt}")
            gi_n = [0]

            def ogroup(lt, c0, c1, eng, dma, tag):
                gi = gi_n[0]
                gi_n[0] += 1
                ops = ps.tile([128, c1 - c0], f32, name=f"ops{gi}",
                              tag=tag, bufs=2)
                lsl = slice(lt * 128, (lt + 1) * 128)
                for o in range(SO):
                    nc.tensor.matmul(ops, cT_sb[:, o, lsl],
                                     wo_sb[:, o, c0:c1],
                                     start=(o == 0), stop=(o == SO - 1))
                ot = ot_tiles[lt]
                if eng == "act":
                    nc.scalar.activation(ot[:, c0:c1], ops, IDENT)
                else:
                    nc.vector.tensor_copy(ot[:, c0:c1], ops)
                if dma == "piece":
                    nc.sync.dma_start(out[lt, :, c0:c1], ot[:, c0:c1])
                elif dma == "full":
                    nc.sync.dma_start(out[lt, :, :], ot)
                elif dma == "tail":
                    # covers all four [128]-col groups; waits all copies
                    nc.sync.dma_start(out[lt, :, 512:1024], ot[:, 512:1024])

            for hp in range(2):
                for lt in range(SO):
                    phase2_unit(3, pt, hp, lt, c3)
            for lt in range(SO):
                transpose_lt(3, lt, c3)
            ogroup(3, 0, 512, "act", "piece", "proj")
            for lt in range(3):
                # lt2 ships as two piece-DMAs: its high half (the last big
                # transfer before the merged tail DMA) is then 364ns instead
                # of 790, letting the tail transfer start earlier; its copy
                # goes to Act so the piece-DMA's SEQ hold starts (and ends)
                # before the merged tail DMA needs the queue
                ogroup(lt, 0, 512, "act",
                       "piece" if lt == 2 else None, "st")
                ogroup(lt, 512, 1024, "dve",
                       "piece" if lt == 2 else "full", "proj")
            for i in range(4):
                ogroup(3, 512 + 128 * i, 640 + 128 * i,
                       ("act", "dve")[i % 2],
                       "tail" if i == 3 else None,
                       ("st", "ctx", "proj", "ctx")[i])

    return _split_multi_waits(nc)


_NC = None


def _get_nc():
    global _NC
    if _NC is None:
        _NC = _build()
    return _NC


def _shard(q, kv, Wq, bq, Wk, bk, Wv, bv, Wo, bo):
    b16 = ml_dtypes.bfloat16

    def lay(a2d, co):  # [co*128, F] -> [128, co, F]
        F = a2d.shape[1]
        return np.ascontiguousarray(
            a2d.reshape(co, 128, F).transpose(1, 0, 2)).astype(b16)

    idn = np.eye(128, dtype=b16)
    in_maps = []
    for core in range(8):
        b, g = core // 2, core % 2
        sl = slice(g * S, (g + 1) * S)
        wq_l = lay(np.ascontiguousarray((Wq[sl] * 0.125).T), CO)
        wk_l = lay(np.ascontiguousarray(Wk[sl].T), CO)
        m = {
            "wqk0": np.ascontiguousarray(
                np.concatenate([wq_l[:, :, 0:128], wk_l[:, :, 0:128]],
                               axis=2)),
            "wqkr": np.ascontiguousarray(
                np.concatenate([wq_l[:, :, 128:512], wk_l[:, :, 128:512]],
                               axis=2)),
            "qT": lay(np.ascontiguousarray(q[b].T), CO),
            "kvT": lay(np.ascontiguousarray(kv[b].T), CO),
            "wvT": lay(np.ascontiguousarray(Wv[sl].T), CO),
            "woT": lay(np.ascontiguousarray(Wo[:, sl].T), SO),
            "bq": np.ascontiguousarray(
                (bq[sl] * 0.125).reshape(SO, 128).T).astype(np.float32),
            "ident": idn,
        }
        in_maps.append(m)
    return in_maps


def _run(in_maps, trace=False):
    res = run_bass_kernel_spmd(_get_nc(), in_maps, core_ids=list(range(8)),
                               trace=trace)
    return res


def kernel(q, kv, Wq, bq, Wk, bk, Wv, bv, Wo, bo, _trace=False):
    q, kv = np.asarray(q, np.float32), np.asarray(kv, np.float32)
    Wq, Wk = np.asarray(Wq, np.float32), np.asarray(Wk, np.float32)
    Wv, Wo = np.asarray(Wv, np.float32), np.asarray(Wo, np.float32)
    bq, bk = np.asarray(bq, np.float32), np.asarray(bk, np.float32)
    bv, bo = np.asarray(bv, np.float32), np.asarray(bo, np.float32)

    in_maps = _shard(q, kv, Wq, bq, Wk, bk, Wv, bv, Wo, bo)
    res = _run(in_maps, trace=_trace)
    B = q.shape[0]
    # bv commutes through the softmax average; bk cancels in softmax.
    const_row = bv @ Wo.T + bo
    outp = np.empty((B, LQ, D), np.float32)
    for b in range(B):
        p0 = np.asarray(res.results[2 * b]["out"],
                        np.float32).reshape(LQ, D)
        p1 = np.asarray(res.results[2 * b + 1]["out"],
                        np.float32).reshape(LQ, D)
        outp[b] = p0 + p1 + const_row[None, :]
    if _trace:
        kernel._last_exec_ns = res.exec_time_ns
        kernel._last_trace = res.instructions_and_trace
    return outp
